# revision 25
# baseline (speedup 1.0000x reference)
"""CLCE loss kernel for Trainium2 (8 NeuronCores, SPMD) — symmetric version.

Loss = 0.5 * cl + 0.5 * ce,
  cl_i = log(exp(slot0_i) + (T_i - P_i) + (2N-2 - num_neg_i)) - slot0_i
  T_i  = sum_j exp((xn_i . xn_j + 1) * 0.25)   <- device
  P_i, slot0_i, ce                              <- host (small / O(N*C))

exp(sim) is symmetric, so each 128x128 tile pair is computed once:
row-tile a's block row covers (a, b) for b <= a; the mirror (b, a) is
recovered as column sums of the exp'ed block (DVE partition folds).

Uniform 20-item program (identical instruction stream on all cores):
4 stationary slots with chunk-prefix bindings L = [2, 4, 6, 8]; slot s
on core c holds row-tile [c, c+8, 23-c, 31-c][s].  Item (h, s) computes
exp(sim[slot_s rows, chunk h]) for h < L[s].  Items where h > r//4 are
garbage (duplicate work, ignored); in the diagonal item (h == r//4) only
the first r%4+1 row-sum segments / r%4 col-sum segments are valid.  The
host masks via the per-item segmented outputs.  PE work: 20*2048 cycles
= 17.1us/core vs 65536 cycles = 27.3us for the full row block.
"""

import os
from contextlib import ExitStack

import numpy as np

import concourse.bass as bass
import concourse.tile as tile
from concourse import bacc, mybir
from concourse.bass_utils import run_bass_kernel_spmd

N, D, C = 4096, 1024, 512
TAU = 0.5
LAMBD = 0.5
NCORES = 8
P = 128
KT = D // 256              # 4 DoubleRow contraction super-tiles
W2 = 512                   # chunk width (1 psum bank)
HC2 = N // W2              # 8 chunks
MT = 4                     # row-tiles (slots) per core
S8 = 16.0                  # fp8 pre-scale for the embeddings

L = [2, 4, 6, 8]           # chunk-prefix binding per slot
ITEMS = [(h, s) for h in range(HC2) for s in range(MT) if L[s] > h]
NI = len(ITEMS)            # 20

_F32 = mybir.dt.float32
_BF16 = mybir.dt.bfloat16
_FP8 = mybir.dt.float8e4
_EXP = mybir.ActivationFunctionType.Exp
_DR = mybir.MatmulPerfMode.DoubleRow
_ADD = mybir.AluOpType.add


def _build_kernel(tc, st, xh, out_rs, out_cs):
    """st:  [P, MT*KT*2*P] fp8   4 stationary slots, k-major per partition
    xh:  [P, HC2*KT*2*W2] fp8  8 chunks, k-major per partition
    out_rs: [P, NI] f32        per-item unsegmented row sums (accum_out)
    out_cs: [1, NI*W2] f32     per-item column sums (ones-matmul on PE)

    Row sums ride the exp activation's accum_out for free; within-chunk
    mirror pairs are covered by the two rows' own (overlapping) row sums,
    so column sums only matter for items with h < r//4 (host masks).
    Column sums come from a ones-stationary matmul over the exp'ed block
    (+512 PE cycles/item); the DVE only copies [1, W2] psum rows out.
    """
    nc = tc.nc
    with ExitStack() as ctx:
        pers = ctx.enter_context(tc.tile_pool(name="pers", bufs=1))
        epool = ctx.enter_context(tc.tile_pool(name="epool", bufs=3))
        psum = ctx.enter_context(
            tc.tile_pool(name="psum", bufs=4, space=bass.MemorySpace.PSUM)
        )
        cspsum = ctx.enter_context(
            tc.tile_pool(name="cspsum", bufs=2, space=bass.MemorySpace.PSUM)
        )

        SS = [
            pers.tile([P, KT, 2, P], _FP8, name=f"ss{s}", tag=f"ss{s}")
            for s in range(MT)
        ]
        XH = [
            pers.tile([P, KT, 2, W2], _FP8, name=f"xh{h}", tag=f"xh{h}")
            for h in range(HC2)
        ]
        RS = pers.tile([P, NI], _F32)
        CS1 = pers.tile([1, NI * W2], _F32)
        ONES = pers.tile([P, 1], _BF16)
        bias_s = pers.tile([P, 1], _F32)
        bias_z = pers.tile([P, 1], _F32)
        warm = pers.tile([P, 1], _F32)
        ZW = pers.tile([P, 512], _BF16)

        nc.gpsimd.memset(ZW[:], 0.0)
        nc.gpsimd.memset(ONES[:], 1.0)
        nc.gpsimd.memset(CS1[:], 0.0)   # skipped items' slots stay zero
        nc.gpsimd.memset(bias_s[:], 0.5 * TAU)
        nc.gpsimd.memset(bias_z[:], 0.0)
        nc.scalar.activation(warm[:], bias_z[:], _EXP, bias=bias_z[:], scale=1.0)

        # --- input DMAs, consumption order, all on the sync (qSP) ring ---
        st5 = st.rearrange("p (s k i n) -> p s k i n", s=MT, k=KT, i=2)
        xh5 = xh.rearrange("p (h k i n) -> p h k i n", h=HC2, k=KT, i=2)
        nc.sync.dma_start(SS[0][:], st5[:, 0])
        nc.sync.dma_start(XH[0][:], xh5[:, 0])
        for s in range(1, MT):
            nc.sync.dma_start(SS[s][:], st5[:, s])
        for h in range(1, HC2):
            nc.sync.dma_start(XH[h][:], xh5[:, h])

        # PE warm-up: hold the PE busy until the first operands land.  A
        # longer run also flips the HAM clock gate to full speed earlier
        # (12 warm-ups empirically ramp it ~4us sooner than 6).
        wps = psum.tile([P, W2], _F32, tag="ps")
        for _ in range(12):
            nc.tensor.matmul(wps[:, 0:512], ZW[:, 0:P], ZW[:], start=True, stop=True)

        act_scale = 0.5 * TAU / (S8 * S8)

        # software-pipelined: item i's colsum matmul is emitted two items'
        # sim matmuls later so the PE never waits on the exp (ScalarE).
        pend = []  # (e tile, item index) awaiting colsum

        def colsum(e, i):
            cp = cspsum.tile([1, W2], _F32, tag="cs")
            nc.tensor.matmul(cp[:], ONES[:], e[:], start=True, stop=True)
            nc.vector.tensor_copy(CS1[:, i * W2:(i + 1) * W2], cp[:])

        for i, (h, s) in enumerate(ITEMS):
            ps = psum.tile([P, W2], _F32, tag="ps")
            for k in range(KT):
                nc.tensor.matmul(
                    ps[:],
                    SS[s][:, k, :, :],
                    XH[h][:, k, :, :],
                    start=(k == 0),
                    stop=(k == KT - 1),
                    perf_mode=_DR,
                )
            if len(pend) >= 2:
                colsum(*pend.pop(0))
            e = epool.tile([P, W2], _BF16)
            nc.scalar.activation(
                e[:], ps[:], _EXP, bias=bias_s[:], scale=act_scale,
                accum_out=RS[:, i:i + 1],
            )
            if h < L[s] - 1:
                # the last-chunk item of every slot is diagonal-or-garbage
                # on every core: its colsum is never read -> skip it
                pend.append((e, i))
            if i == 14:
                # early out-DMA: finished items' sums ship mid-stream
                nc.sync.dma_start(out_cs[:, 0:13 * W2], CS1[:, 0:13 * W2])
                nc.sync.dma_start(out_rs[:, 0:14], RS[:, 0:14])

        for pe in pend:
            colsum(*pe)
        nc.sync.dma_start(out_cs[:, 13 * W2:], CS1[:, 13 * W2:])
        nc.scalar.dma_start(out_rs[:, 14:], RS[:, 14:])


_NC_CACHE = None


def _get_nc():
    global _NC_CACHE
    if _NC_CACHE is None:
        nc = bacc.Bacc(
            "TRN2", target_bir_lowering=False, debug=False,
            enable_asserts=False, num_devices=NCORES,
        )
        st_d = nc.dram_tensor("st", [P, MT * KT * 2 * P], _FP8, kind="ExternalInput")
        xh_d = nc.dram_tensor("xh", [P, HC2 * KT * 2 * W2], _FP8, kind="ExternalInput")
        rs_d = nc.dram_tensor("rs", [P, NI], _F32, kind="ExternalOutput")
        cs_d = nc.dram_tensor("cs", [1, NI * W2], _F32, kind="ExternalOutput")
        with tile.TileContext(nc) as tc:
            _build_kernel(tc, st_d.ap(), xh_d.ap(), rs_d.ap(), cs_d.ap())
        nc.compile()
        _NC_CACHE = nc
    return _NC_CACHE


def _pack_fp8(zT, cols):
    """[D, ncols] f32 -> [KT*P, 2*ncols] fp8, DoubleRow pairing."""
    fp8np = mybir.dt.np(_FP8)
    q = zT.reshape(KT, 2, P, cols).transpose(0, 2, 1, 3).reshape(KT * P, 2 * cols)
    return np.ascontiguousarray(q.astype(fp8np))


def _rows_for_core(c):
    return [c, c + 8, 23 - c, 31 - c]


def _run_device(xnT, trace=False):
    """Run the SPMD kernel; returns T[N] f64 and the raw results."""
    zT = (xnT * S8).astype(np.float32)  # [D, N]
    xt8 = _pack_fp8(zT, N)              # [(k p), (i n)]
    # chunk-major, partition-row layout: xh[p, h, k, i, n]
    xh_host = np.ascontiguousarray(
        xt8.reshape(KT, P, 2, HC2, W2).transpose(1, 3, 0, 2, 4)
        .reshape(P, HC2 * KT * 2 * W2)
    )
    in_maps = []
    for c in range(NCORES):
        st_host = np.empty((P, MT * KT * 2 * P), xt8.dtype)
        for s, r in enumerate(_rows_for_core(c)):
            blockq = _pack_fp8(
                np.ascontiguousarray(zT[:, r * P:(r + 1) * P]), P
            )  # [(k p), (i n)] = [512, 256]
            st_host[:, s * KT * 2 * P:(s + 1) * KT * 2 * P] = (
                blockq.reshape(KT, P, 2 * P).transpose(1, 0, 2).reshape(P, KT * 2 * P)
            )
        in_maps.append({"st": np.ascontiguousarray(st_host), "xh": xh_host})
    res = run_bass_kernel_spmd(
        _get_nc(), in_maps, core_ids=list(range(NCORES)), trace=trace,
    )
    T = np.zeros(N, np.float64)
    for c, r_ in enumerate(res.results):
        rs = r_["rs"].astype(np.float64)        # [128, NI]
        cs = r_["cs"].astype(np.float64)[0]     # [NI*W2]
        for i, (h, s) in enumerate(ITEMS):
            r = _rows_for_core(c)[s]
            q = r // 4
            if h > q:
                continue
            rows = slice(r * P, (r + 1) * P)
            T[rows] += rs[:, i]
            if h < q:
                cols = slice(h * W2, (h + 1) * W2)
                T[cols] += cs[i * W2:(i + 1) * W2]
    return T, res


def kernel(layer_embeds, y_true, y_pred):
    x = np.asarray(layer_embeds, dtype=np.float32)
    yt = np.asarray(y_true).astype(np.int64)
    yp = np.asarray(y_pred, dtype=np.float32)

    norms = np.maximum(
        np.sqrt((x.astype(np.float64) ** 2).sum(1, keepdims=True)), 1e-8
    )
    xn = (x / norms).astype(np.float32)
    xnT = np.ascontiguousarray(xn.T)  # [D, N]

    trace = bool(int(os.environ.get("CLCE_TRACE", "0")))
    T, res = _run_device(xnT, trace=trace)
    if trace:
        kernel.last_results = res

    # --- host-side small terms ---
    fp8np = mybir.dt.np(_FP8)
    xq = (xn * S8).astype(fp8np).astype(np.float64) / S8
    counts = np.bincount(yt, minlength=C)
    P_ = np.zeros(N, np.float64)
    slot0 = np.zeros(N, np.float64)
    for cval in np.unique(yt):
        idx = np.where(yt == cval)[0]
        subq = xq[idx]
        sq = (subq @ subq.T + 1.0) * (0.5 * TAU)
        P_[idx] = np.exp(sq).sum(1)
        if len(idx) >= 2:
            sub = xn[idx].astype(np.float64)
            s = (sub @ sub.T + 1.0) * (0.5 * TAU)
            firstpos = np.where(np.arange(len(idx)) == 0, 1, 0)
            slot0[idx] = s[np.arange(len(idx)), firstpos]

    num_neg = N - counts[yt]
    S = T - P_
    Z = (2 * N - 2 - num_neg).astype(np.float64)
    cl = (np.log(np.exp(slot0) + S + Z) - slot0).mean()

    # cross-entropy fully on host (f64)
    ypd = yp.astype(np.float64)
    m = ypd.max(axis=1, keepdims=True)
    lse = np.log(np.exp(ypd - m).sum(axis=1)) + m[:, 0]
    ce = (lse - ypd[np.arange(N), yt]).mean()

    loss = LAMBD * cl + (1.0 - LAMBD) * ce
    return np.asarray(loss, dtype=np.float32)


# revision 28
# speedup vs baseline: 1.0125x; 1.0125x over previous
"""CLCE loss kernel for Trainium2 (8 NeuronCores, SPMD) — symmetric version.

Loss = 0.5 * cl + 0.5 * ce,
  cl_i = log(exp(slot0_i) + (T_i - P_i) + (2N-2 - num_neg_i)) - slot0_i
  T_i  = sum_j exp((xn_i . xn_j + 1) * 0.25)   <- device
  P_i, slot0_i, ce                              <- host (small / O(N*C))

exp(sim) is symmetric, so each 128x128 tile pair is computed once:
row-tile a's block row covers (a, b) for b <= a; the mirror (b, a) is
recovered as column sums of the exp'ed block (DVE partition folds).

Uniform 20-item program (identical instruction stream on all cores):
4 stationary slots with chunk-prefix bindings L = [2, 4, 6, 8]; slot s
on core c holds row-tile [c, c+8, 23-c, 31-c][s].  Item (h, s) computes
exp(sim[slot_s rows, chunk h]) for h < L[s].  Items where h > r//4 are
garbage (duplicate work, ignored); in the diagonal item (h == r//4) only
the first r%4+1 row-sum segments / r%4 col-sum segments are valid.  The
host masks via the per-item segmented outputs.  PE work: 20*2048 cycles
= 17.1us/core vs 65536 cycles = 27.3us for the full row block.
"""

import os
from contextlib import ExitStack

import numpy as np

import concourse.bass as bass
import concourse.tile as tile
from concourse import bacc, mybir
from concourse.bass_utils import run_bass_kernel_spmd

N, D, C = 4096, 1024, 512
TAU = 0.5
LAMBD = 0.5
NCORES = 8
P = 128
KT = D // 256              # 4 DoubleRow contraction super-tiles
W2 = 512                   # chunk width (1 psum bank)
HC2 = N // W2              # 8 chunks
MT = 4                     # row-tiles (slots) per core
S8 = 16.0                  # fp8 pre-scale for the embeddings

L = [2, 4, 6, 8]           # chunk-prefix binding per slot
ITEMS = [(h, s) for h in range(HC2) for s in range(MT) if L[s] > h]
# tail reorder: end on no-colsum items ((5,2) and (7,3)) so the last
# needed colsum+copy overlap the final sim matmuls instead of trailing
ITEMS[16], ITEMS[17], ITEMS[18] = ITEMS[17], ITEMS[18], ITEMS[16]
NI = len(ITEMS)            # 20

_F32 = mybir.dt.float32
_BF16 = mybir.dt.bfloat16
_FP8 = mybir.dt.float8e4
_EXP = mybir.ActivationFunctionType.Exp
_DR = mybir.MatmulPerfMode.DoubleRow
_ADD = mybir.AluOpType.add


def _build_kernel(tc, st, xh, out_rs, out_cs):
    """st:  [P, MT*KT*2*P] fp8   4 stationary slots, k-major per partition
    xh:  [P, HC2*KT*2*W2] fp8  8 chunks, k-major per partition
    out_rs: [P, NI] f32        per-item unsegmented row sums (accum_out)
    out_cs: [1, NI*W2] f32     per-item column sums (ones-matmul on PE)

    Row sums ride the exp activation's accum_out for free; within-chunk
    mirror pairs are covered by the two rows' own (overlapping) row sums,
    so column sums only matter for items with h < r//4 (host masks).
    Column sums come from a ones-stationary matmul over the exp'ed block
    (+512 PE cycles/item); the DVE only copies [1, W2] psum rows out.
    """
    nc = tc.nc
    with ExitStack() as ctx:
        pers = ctx.enter_context(tc.tile_pool(name="pers", bufs=1))
        epool = ctx.enter_context(tc.tile_pool(name="epool", bufs=3))
        psum = ctx.enter_context(
            tc.tile_pool(name="psum", bufs=4, space=bass.MemorySpace.PSUM)
        )
        cspsum = ctx.enter_context(
            tc.tile_pool(name="cspsum", bufs=2, space=bass.MemorySpace.PSUM)
        )

        SS = [
            pers.tile([P, KT, 2, P], _FP8, name=f"ss{s}", tag=f"ss{s}")
            for s in range(MT)
        ]
        XH = [
            pers.tile([P, KT, 2, W2], _FP8, name=f"xh{h}", tag=f"xh{h}")
            for h in range(HC2)
        ]
        RS = pers.tile([P, NI], _F32)
        CS1 = pers.tile([1, NI * W2], _F32)
        ONES = pers.tile([P, 1], _BF16)
        bias_s = pers.tile([P, 1], _F32)
        bias_z = pers.tile([P, 1], _F32)
        warm = pers.tile([P, 1], _F32)
        ZW = pers.tile([P, 512], _BF16)

        nc.gpsimd.memset(ZW[:], 0.0)
        nc.gpsimd.memset(ONES[:], 1.0)
        nc.gpsimd.memset(CS1[:], 0.0)   # skipped items' slots stay zero
        nc.gpsimd.memset(bias_s[:], 0.5 * TAU)
        nc.gpsimd.memset(bias_z[:], 0.0)
        nc.scalar.activation(warm[:], bias_z[:], _EXP, bias=bias_z[:], scale=1.0)

        # --- input DMAs, consumption order, all on the sync (qSP) ring ---
        st5 = st.rearrange("p (s k i n) -> p s k i n", s=MT, k=KT, i=2)
        xh5 = xh.rearrange("p (h k i n) -> p h k i n", h=HC2, k=KT, i=2)
        nc.sync.dma_start(SS[0][:], st5[:, 0])
        nc.sync.dma_start(XH[0][:], xh5[:, 0])
        for s in range(1, MT):
            nc.sync.dma_start(SS[s][:], st5[:, s])
        for h in range(1, HC2):
            nc.sync.dma_start(XH[h][:], xh5[:, h])

        # PE warm-up: hold the PE busy until the first operands land.  A
        # longer run also flips the HAM clock gate to full speed earlier
        # (12 warm-ups empirically ramp it ~4us sooner than 6).
        wps = psum.tile([P, W2], _F32, tag="ps")
        for _ in range(12):
            nc.tensor.matmul(wps[:, 0:512], ZW[:, 0:P], ZW[:], start=True, stop=True)

        act_scale = 0.5 * TAU / (S8 * S8)

        # software-pipelined: item i's colsum matmul is emitted two items'
        # sim matmuls later so the PE never waits on the exp (ScalarE).
        pend = []  # (e tile, item index) awaiting colsum

        def colsum(e, i):
            cp = cspsum.tile([1, W2], _F32, tag="cs")
            nc.tensor.matmul(cp[:], ONES[:], e[:], start=True, stop=True)
            nc.vector.tensor_copy(CS1[:, i * W2:(i + 1) * W2], cp[:])

        for i, (h, s) in enumerate(ITEMS):
            if i == NI - 1:
                # flush before the last item's sims: the copy then overlaps
                # them and the out-DMA can start at the stream's end
                while pend:
                    colsum(*pend.pop(0))
            ps = psum.tile([P, W2], _F32, tag="ps")
            for k in range(KT):
                nc.tensor.matmul(
                    ps[:],
                    SS[s][:, k, :, :],
                    XH[h][:, k, :, :],
                    start=(k == 0),
                    stop=(k == KT - 1),
                    perf_mode=_DR,
                )
            if len(pend) >= 2:
                colsum(*pend.pop(0))
            e = epool.tile([P, W2], _BF16)
            nc.scalar.activation(
                e[:], ps[:], _EXP, bias=bias_s[:], scale=act_scale,
                accum_out=RS[:, i:i + 1],
            )
            if h < L[s] - 1:
                # the last-chunk item of every slot is diagonal-or-garbage
                # on every core: its colsum is never read -> skip it
                pend.append((e, i))
            if i == 14:
                # early out-DMA: finished items' sums ship mid-stream
                nc.sync.dma_start(out_cs[:, 0:13 * W2], CS1[:, 0:13 * W2])
                nc.sync.dma_start(out_rs[:, 0:14], RS[:, 0:14])

        for pe in pend:
            colsum(*pe)
        nc.sync.dma_start(out_cs[:, 13 * W2:], CS1[:, 13 * W2:])
        nc.scalar.dma_start(out_rs[:, 14:NI - 1], RS[:, 14:NI - 1])
        nc.scalar.dma_start(out_rs[:, NI - 1:], RS[:, NI - 1:])


_NC_CACHE = None


def _get_nc():
    global _NC_CACHE
    if _NC_CACHE is None:
        nc = bacc.Bacc(
            "TRN2", target_bir_lowering=False, debug=False,
            enable_asserts=False, num_devices=NCORES,
        )
        st_d = nc.dram_tensor("st", [P, MT * KT * 2 * P], _FP8, kind="ExternalInput")
        xh_d = nc.dram_tensor("xh", [P, HC2 * KT * 2 * W2], _FP8, kind="ExternalInput")
        rs_d = nc.dram_tensor("rs", [P, NI], _F32, kind="ExternalOutput")
        cs_d = nc.dram_tensor("cs", [1, NI * W2], _F32, kind="ExternalOutput")
        with tile.TileContext(nc) as tc:
            _build_kernel(tc, st_d.ap(), xh_d.ap(), rs_d.ap(), cs_d.ap())
        nc.compile()
        _NC_CACHE = nc
    return _NC_CACHE


def _pack_fp8(zT, cols):
    """[D, ncols] f32 -> [KT*P, 2*ncols] fp8, DoubleRow pairing."""
    fp8np = mybir.dt.np(_FP8)
    q = zT.reshape(KT, 2, P, cols).transpose(0, 2, 1, 3).reshape(KT * P, 2 * cols)
    return np.ascontiguousarray(q.astype(fp8np))


def _rows_for_core(c):
    return [c, c + 8, 23 - c, 31 - c]


def _run_device(xnT, trace=False):
    """Run the SPMD kernel; returns T[N] f64 and the raw results."""
    zT = (xnT * S8).astype(np.float32)  # [D, N]
    xt8 = _pack_fp8(zT, N)              # [(k p), (i n)]
    # chunk-major, partition-row layout: xh[p, h, k, i, n]
    xh_host = np.ascontiguousarray(
        xt8.reshape(KT, P, 2, HC2, W2).transpose(1, 3, 0, 2, 4)
        .reshape(P, HC2 * KT * 2 * W2)
    )
    in_maps = []
    for c in range(NCORES):
        st_host = np.empty((P, MT * KT * 2 * P), xt8.dtype)
        for s, r in enumerate(_rows_for_core(c)):
            blockq = _pack_fp8(
                np.ascontiguousarray(zT[:, r * P:(r + 1) * P]), P
            )  # [(k p), (i n)] = [512, 256]
            st_host[:, s * KT * 2 * P:(s + 1) * KT * 2 * P] = (
                blockq.reshape(KT, P, 2 * P).transpose(1, 0, 2).reshape(P, KT * 2 * P)
            )
        in_maps.append({"st": np.ascontiguousarray(st_host), "xh": xh_host})
    res = run_bass_kernel_spmd(
        _get_nc(), in_maps, core_ids=list(range(NCORES)), trace=trace,
    )
    T = np.zeros(N, np.float64)
    for c, r_ in enumerate(res.results):
        rs = r_["rs"].astype(np.float64)        # [128, NI]
        cs = r_["cs"].astype(np.float64)[0]     # [NI*W2]
        for i, (h, s) in enumerate(ITEMS):
            r = _rows_for_core(c)[s]
            q = r // 4
            if h > q:
                continue
            rows = slice(r * P, (r + 1) * P)
            T[rows] += rs[:, i]
            if h < q:
                cols = slice(h * W2, (h + 1) * W2)
                T[cols] += cs[i * W2:(i + 1) * W2]
    return T, res


def kernel(layer_embeds, y_true, y_pred):
    x = np.asarray(layer_embeds, dtype=np.float32)
    yt = np.asarray(y_true).astype(np.int64)
    yp = np.asarray(y_pred, dtype=np.float32)

    norms = np.maximum(
        np.sqrt((x.astype(np.float64) ** 2).sum(1, keepdims=True)), 1e-8
    )
    xn = (x / norms).astype(np.float32)
    xnT = np.ascontiguousarray(xn.T)  # [D, N]

    trace = bool(int(os.environ.get("CLCE_TRACE", "0")))
    T, res = _run_device(xnT, trace=trace)
    if trace:
        kernel.last_results = res

    # --- host-side small terms ---
    fp8np = mybir.dt.np(_FP8)
    xq = (xn * S8).astype(fp8np).astype(np.float64) / S8
    counts = np.bincount(yt, minlength=C)
    P_ = np.zeros(N, np.float64)
    slot0 = np.zeros(N, np.float64)
    for cval in np.unique(yt):
        idx = np.where(yt == cval)[0]
        subq = xq[idx]
        sq = (subq @ subq.T + 1.0) * (0.5 * TAU)
        P_[idx] = np.exp(sq).sum(1)
        if len(idx) >= 2:
            sub = xn[idx].astype(np.float64)
            s = (sub @ sub.T + 1.0) * (0.5 * TAU)
            firstpos = np.where(np.arange(len(idx)) == 0, 1, 0)
            slot0[idx] = s[np.arange(len(idx)), firstpos]

    num_neg = N - counts[yt]
    S = T - P_
    Z = (2 * N - 2 - num_neg).astype(np.float64)
    cl = (np.log(np.exp(slot0) + S + Z) - slot0).mean()

    # cross-entropy fully on host (f64)
    ypd = yp.astype(np.float64)
    m = ypd.max(axis=1, keepdims=True)
    lse = np.log(np.exp(ypd - m).sum(axis=1)) + m[:, 0]
    ce = (lse - ypd[np.arange(N), yt]).mean()

    loss = LAMBD * cl + (1.0 - LAMBD) * ce
    return np.asarray(loss, dtype=np.float32)
